# revision 1
# baseline (speedup 1.0000x reference)
"""Trainium2 Bass kernel for: conv3x3 -> conv3x3 -> maxpool2x2 -> conv3x3 -> conv3x3
on a [1,1,8192,8192] fp32 image, SAME padding, single channel.

Strategy (8 NeuronCores, height-sharded, halo replicated on host — no collectives):
  * conv1*conv2 are composed into one 5x5 correlation ("stage A"); likewise
    conv3*conv4 ("stage B"). Each 5x5 is computed as 5 PSUM-accumulated
    TensorE band matmuls: the stationary [K<=128, M<=124] band matrix carries
    the 5 vertical taps (mapping input rows on partitions -> output rows),
    and the 5 horizontal taps come from shifting the moving operand's column
    window by t=0..4.
  * Fusing two SAME convs is NOT a plain 5x5 at the image border (the
    reference zeroes the intermediate ring). All corrections are folded into
    the band-matrix *data*: edge-row edits in the main bands, plus per-block
    single-column correction matmuls (bandL/bandR) for the left/right image
    columns, with corner fix-ups. Per-core variants also zero the phantom
    pooled halo rows. The SPMD program is identical on all 8 cores; only the
    band-matrix values differ per core.
  * maxpool2x2: stage-A bands write even/odd output rows to separate
    partition groups, so the row-pair max is a plain partition-sliced
    tensor_tensor max; the column-pair max uses stride-2 access patterns.
    Pooled rows are assembled directly into SBUF-resident stage-B input
    tiles; stage B never touches HBM for its input.
"""

import numpy as np

try:
    import concourse.bass  # noqa: F401
except ImportError:
    import sys
    sys.path.insert(0, "/opt/trn_rl_repo")

H = 8192
W = 8192
NCORES = 8
RPC = H // NCORES          # x rows per core
OPC = RPC // 2             # output rows per core
NZ = RPC + 8               # stage-A output rows computed per core
BM = 124                   # output rows per band-matmul block
NBLK = (NZ + BM - 1) // BM           # 9 stage-A blocks
NBB = (OPC + BM - 1) // BM           # 5 stage-B blocks
WH = W // 2
NSTRIPE = 2
SW = W // NSTRIPE          # output cols per stage-A stripe
CH = 512                   # psum chunk width
NCHUNK_A = SW // CH        # 8
NCHUNK_B = WH // CH        # 8
XROWS = RPC + 16
XCOLS = W + 4
DT_F32 = None  # filled lazily (mybir.dt.float32)


# ------------------------------------------------------------------ bands ---

def _conv_full2d(a, b):
    na, ma = a.shape
    nb, mb = b.shape
    out = np.zeros((na + nb - 1, ma + mb - 1), dtype=np.float64)
    for i in range(na):
        for j in range(ma):
            out[i : i + nb, j : j + mb] += a[i, j] * b
    return out


def _stage_consts(w1, w2):
    w1 = np.asarray(w1, np.float64)
    w2 = np.asarray(w2, np.float64)
    return dict(
        K5=_conv_full2d(w1, w2),
        kh0=np.convolve(w2[0, :], w1[2, :]),
        khb=np.convolve(w2[2, :], w1[0, :]),
        kv0=np.convolve(w2[:, 0], w1[:, 2]),
        kvW=np.convolve(w2[:, 2], w1[:, 0]),
        c00=w2[0, 0] * w1[2, 2],
        c0W=w2[0, 2] * w1[2, 0],
        cH0=w2[2, 0] * w1[0, 2],
        cHW=w2[2, 2] * w1[0, 0],
    )


def _rowmap_permuted(M):
    h = M // 2
    return np.array([2 * m if m < h else 2 * (m - h) + 1 for m in range(M)])


def _build_stage_bands(C, K, M, rowmap, glob_rows, Hout, zero_rows=()):
    """bands [5][K, M], bandL [K, M], bandR [K, M] (float64)."""
    bands = np.zeros((5, K, M), dtype=np.float64)
    bandL = np.zeros((K, M), dtype=np.float64)
    bandR = np.zeros((K, M), dtype=np.float64)
    for m in range(M):
        r = rowmap[m]
        for a in range(5):
            k = r + a
            if k >= K:
                continue
            bands[:, k, m] = C["K5"][a, :]
            bandL[k, m] = -C["kv0"][a]
            bandR[k, m] = -C["kvW"][a]
        g = glob_rows[m]
        k2 = r + 2
        if k2 < K:
            if g == 0:
                bands[:, k2, m] -= C["kh0"]
                bandL[k2, m] += C["c00"]
                bandR[k2, m] += C["c0W"]
            if g == Hout - 1:
                bands[:, k2, m] -= C["khb"]
                bandL[k2, m] += C["cH0"]
                bandR[k2, m] += C["cHW"]
    for k in zero_rows:
        bands[:, k, :] = 0.0
        bandL[k, :] = 0.0
        bandR[k, :] = 0.0
    return bands, bandL, bandR


def _pack(bands):
    """[5, K, M] -> [K, 5*M] matching lhsT slices [K, t*M:(t+1)*M]."""
    b, K, M = bands.shape
    return np.ascontiguousarray(
        bands.transpose(1, 0, 2).reshape(K, b * M).astype(np.float32)
    )


def _scatter_m128(bands, bandL, bandR, Mz):
    """Place dense output cols into the 128-wide layout: even rows at
    partitions 0..h-1, odd rows at 64..64+h-1 (rest zero)."""
    h = Mz // 2
    K = bands.shape[1]
    b128 = np.zeros((5, K, 128), dtype=np.float64)
    l128 = np.zeros((K, 128), dtype=np.float64)
    r128 = np.zeros((K, 128), dtype=np.float64)
    for m in range(Mz):
        col = m if m < h else 64 + (m - h)
        b128[:, :, col] = bands[:, :, m]
        l128[:, col] = bandL[:, m]
        r128[:, col] = bandR[:, m]
    return b128, l128, r128


def _core_weights(c, w1, w2, w3, w4):
    """All weight arrays for core c, keyed by DRAM tensor name."""
    CA = _stage_consts(w1, w2)
    CB = _stage_consts(w3, w4)
    out = {}
    far = np.full(BM, 10 ** 9)  # glob rows that trigger no edits

    # --- stage A ---
    def bands_a(blk):
        Mz = min(BM, NZ - BM * blk)
        Kx = Mz + 4
        rm = _rowmap_permuted(Mz)
        glob = np.array([RPC * c - 4 + BM * blk + rm[m] for m in range(Mz)])
        return _scatter_m128(*_build_stage_bands(CA, Kx, Mz, rm, glob, H), Mz)

    gen = _scatter_m128(
        *_build_stage_bands(CA, 128, BM, _rowmap_permuted(BM), far, H), BM
    )
    blk0 = bands_a(0)
    blk8 = bands_a(NBLK - 1)
    out["wa"], out["wla"], out["wra"] = _pack(gen[0]), *(
        np.ascontiguousarray(g.astype(np.float32)) for g in gen[1:]
    )
    out["wa0"], out["wla0"], out["wra0"] = _pack(blk0[0]), *(
        np.ascontiguousarray(g.astype(np.float32)) for g in blk0[1:]
    )
    out["wa8"], out["wla8"], out["wra8"] = _pack(blk8[0]), *(
        np.ascontiguousarray(g.astype(np.float32)) for g in blk8[1:]
    )

    # --- stage B ---
    def bands_b(d):
        Mo = min(BM, OPC - BM * d)
        Kp = Mo + 4
        rm = np.arange(Mo)
        glob = np.array([OPC * c + BM * d + m for m in range(Mo)])
        zero = [
            k
            for k in range(Kp)
            if not (0 <= OPC * c - 2 + BM * d + k < H // 2)
        ]
        return _build_stage_bands(CB, Kp, Mo, rm, glob, H // 2, zero)

    genb = _build_stage_bands(CB, 128, BM, np.arange(BM), far, H // 2)
    d0 = bands_b(0)
    d4 = bands_b(NBB - 1)
    out["wb"], out["wlb"], out["wrb"] = _pack(genb[0]), *(
        np.ascontiguousarray(g.astype(np.float32)) for g in genb[1:]
    )
    out["wb0"], out["wlb0"], out["wrb0"] = _pack(d0[0]), *(
        np.ascontiguousarray(g.astype(np.float32)) for g in d0[1:]
    )
    out["wb4"], out["wlb4"], out["wrb4"] = _pack(d4[0]), *(
        np.ascontiguousarray(g.astype(np.float32)) for g in d4[1:]
    )
    return out


def _local_x(x2, c):
    """[XROWS, XCOLS] zero-padded slice for core c (rows RPC*c-6 .., cols -2..)."""
    r0 = RPC * c - 6
    out = np.zeros((XROWS, XCOLS), dtype=_host_dt())
    rlo, rhi = max(r0, 0), min(r0 + XROWS, H)
    out[rlo - r0 : rhi - r0, 2 : 2 + W] = x2[rlo:rhi].astype(_host_dt())
    return out


# ---------------------------------------------------------------- program ---

_PROGRAM = {}


MM_DTYPE = "float32r"  # "float32" (4 cyc/row, exact), "float32r" (1 cyc/row,
#                          ~2.5e-4 rel err), or "bfloat16" (1 cyc/row, ~1e-3)


def _host_dt():
    if MM_DTYPE == "bfloat16":
        import ml_dtypes
        return ml_dtypes.bfloat16
    return np.float32


def _build_program(loop=False):
    import concourse.bacc as bacc
    import concourse.mybir as mybir
    import concourse.tile as tile
    from contextlib import ExitStack

    f32 = mybir.dt.float32
    mmdt = getattr(mybir.dt, MM_DTYPE)

    def mm(ap):
        return ap

    nc = bacc.Bacc("TRN2")

    xdram = nc.dram_tensor("x", [XROWS, XCOLS], mmdt, kind="ExternalInput")
    wshape = {
        "wa": [128, 5 * 128], "wa0": [128, 5 * 128], "wa8": [44, 5 * 128],
        "wla": [128, 128], "wra": [128, 128],
        "wla0": [128, 128], "wra0": [128, 128],
        "wla8": [44, 128], "wra8": [44, 128],
        "wb": [128, 5 * BM], "wb0": [128, 5 * BM], "wb4": [20, 5 * 16],
        "wlb": [128, BM], "wrb": [128, BM],
        "wlb0": [128, BM], "wrb0": [128, BM],
        "wlb4": [20, 16], "wrb4": [20, 16],
    }
    wdram = {
        k: nc.dram_tensor(k, v, mmdt, kind="ExternalInput")
        for k, v in wshape.items()
    }
    zdram = nc.dram_tensor("zpad", [128, 4], mmdt, kind="ExternalInput")
    if loop:
        ndram = nc.dram_tensor("niter", [1, 1], mybir.dt.int32,
                               kind="ExternalInput")
    outdram = nc.dram_tensor("out", [OPC, WH], f32, kind="ExternalOutput")

    with tile.TileContext(nc) as tc:
        with (
            tc.tile_pool(name="wpool", bufs=1) as wpool,
            tc.tile_pool(name="xpool", bufs=2) as xpool,
            tc.tile_pool(name="zpool", bufs=3) as zpool,
            tc.tile_pool(name="hpool", bufs=3) as hpool,
            tc.tile_pool(name="h2pool", bufs=3) as h2pool,
            tc.tile_pool(name="spool", bufs=2) as spool,
            tc.tile_pool(name="ppool", bufs=1) as ppool,
            tc.tile_pool(name="opool", bufs=2) as opool,
            tc.tile_pool(name="zps_pool", bufs=4, space="PSUM") as zps_pool,
            tc.tile_pool(name="bps_pool", bufs=2, space="PSUM") as bps_pool,
            tc.tile_pool(name="cps_pool", bufs=2, space="PSUM") as cps_pool,
        ):
            wt = {}
            for k, shp in wshape.items():
                wt[k] = wpool.tile(shp, mmdt, name=f"wt_{k}")
                nc.sync.dma_start(out=wt[k], in_=wdram[k][:])

            if loop:
                import concourse.mybir as _mb
                nt = wpool.tile([1, 1], _mb.dt.int32, name="nt")
                nc.sync.dma_start(out=nt, in_=ndram[:])
                nval = nc.values_load(nt[0:1, 0:1], min_val=1, max_val=4096)
                loop_cm = tc.For_i(0, nval, 1, name="rep")
            else:
                loop_cm = None

            with (loop_cm if loop_cm is not None else ExitStack()):
                pooled = []
                for d in range(NBB):
                    pt = ppool.tile([128, WH + 4], mmdt, name=f"pooled{d}")
                    pooled.append(pt)
                    nc.sync.dma_start(out=pt[:, 0:2], in_=zdram[:, 0:2])
                    nc.sync.dma_start(out=pt[:, WH + 2 : WH + 4], in_=zdram[:, 2:4])

                # ---------------- stage A ----------------
                for b in range(NBLK):
                    Mz = min(BM, NZ - BM * b)
                    h = Mz // 2
                    Kx = Mz + 4
                    wa_t = wt["wa0"] if b == 0 else (wt["wa8"] if b == NBLK - 1 else wt["wa"])
                    wl_t = wt["wla0"] if b == 0 else (wt["wla8"] if b == NBLK - 1 else wt["wla"])
                    wr_t = wt["wra0"] if b == 0 else (wt["wra8"] if b == NBLK - 1 else wt["wra"])
                    d_t, po = divmod(62 * b, BM)
                    for s in range(NSTRIPE):
                        st = spool.tile([64, SW // 2 + 4], mmdt, name="stg")
                        xt = xpool.tile([128, SW + 4], mmdt, name="xt")
                        nc.sync.dma_start(
                            out=xt[0:Kx],
                            in_=xdram[BM * b : BM * b + Kx, SW * s : SW * s + SW + 4],
                        )
                        for jj in range(NCHUNK_A):
                            zps = zps_pool.tile([128, CH], f32, name="zps")
                            corr = ("L" if (s == 0 and jj == 0) else
                                    "R" if (s == NSTRIPE - 1 and jj == NCHUNK_A - 1) else None)
                            for t in range(5):
                                nc.tensor.matmul(
                                    zps,
                                    lhsT=mm(wa_t[0:Kx, t * 128 : (t + 1) * 128]),
                                    rhs=mm(xt[0:Kx, CH * jj + t : CH * jj + t + CH]),
                                    start=(t == 0),
                                    stop=(t == 4),
                                )
                            cps = None
                            if corr == "L":
                                # rhs col0 = padded zero, col1 = x col 0
                                cps = cps_pool.tile([128, 2], f32, name="cps", tag="cps")
                                nc.tensor.matmul(
                                    cps, lhsT=mm(wl_t[0:Kx]),
                                    rhs=mm(xt[0:Kx, 1:3]), start=True, stop=True,
                                )
                            elif corr == "R":
                                # rhs col0 = x col W-1, col1 = padded zero
                                cps = cps_pool.tile([128, 2], f32, name="cps", tag="cps")
                                nc.tensor.matmul(
                                    cps, lhsT=mm(wr_t[0:Kx]),
                                    rhs=mm(xt[0:Kx, SW + 1 : SW + 3]), start=True, stop=True,
                                )
                            zsb = zpool.tile([128, CH], f32, name="zsb")
                            nc.scalar.copy(out=zsb, in_=zps)
                            if corr == "L":
                                nc.vector.tensor_add(
                                    out=zsb[:, 0:1], in0=zsb[:, 0:1], in1=cps[:, 1:2]
                                )
                            elif corr == "R":
                                nc.vector.tensor_add(
                                    out=zsb[:, CH - 1 : CH], in0=zsb[:, CH - 1 : CH],
                                    in1=cps[:, 0:1],
                                )
                            hp = hpool.tile([128, CH // 2], f32, name="hp")
                            nc.vector.tensor_max(
                                out=hp, in0=zsb[:, 0:CH:2], in1=zsb[:, 1:CH:2]
                            )
                            # row-pair max: TT needs equal base partitions, so
                            # first move the odd-row half down to base 0
                            hp2 = h2pool.tile([64, CH // 2], f32, name="hp2")
                            nc.vector.tensor_copy(out=hp2, in_=hp[64:128])
                            col0 = 2 + (CH // 2) * jj
                            nc.vector.tensor_max(
                                out=st[:, col0 : col0 + CH // 2],
                                in0=hp[0:64],
                                in1=hp2,
                            )
                        # scatter the stripe's pooled rows into the stage-B input
                        # tiles (DMA: compute engines can't address partition 62)
                        pc0 = 2 + (SW // 2) * s
                        nc.sync.dma_start(
                            out=pooled[d_t][po : po + h, pc0 : pc0 + SW // 2],
                            in_=st[0:h, 2 : SW // 2 + 2],
                        )
                        if b >= 2 and b % 2 == 0:
                            # pooled tiles overlap by 4 rows at band boundaries
                            nc.sync.dma_start(
                                out=pooled[b // 2 - 1][BM : BM + 4, pc0 : pc0 + SW // 2],
                                in_=st[0:4, 2 : SW // 2 + 2],
                            )

                # ---------------- stage B ----------------
                for d in range(NBB):
                    Mo = min(BM, OPC - BM * d)
                    Kp = Mo + 4
                    wb_t = wt["wb0"] if d == 0 else (wt["wb4"] if d == NBB - 1 else wt["wb"])
                    wlb_t = wt["wlb0"] if d == 0 else (wt["wlb4"] if d == NBB - 1 else wt["wlb"])
                    wrb_t = wt["wrb0"] if d == 0 else (wt["wrb4"] if d == NBB - 1 else wt["wrb"])
                    for half in range(2):
                        osb = opool.tile([BM, WH // 2], f32, name="osb")
                        for jh in range(NCHUNK_B // 2):
                            jj = half * (NCHUNK_B // 2) + jh
                            bps = bps_pool.tile([BM, CH], f32, name="bps")
                            corr = ("L" if jj == 0 else
                                    "R" if jj == NCHUNK_B - 1 else None)
                            for t in range(5):
                                nc.tensor.matmul(
                                    bps[0:Mo],
                                    lhsT=mm(wb_t[0:Kp, t * Mo : (t + 1) * Mo]),
                                    rhs=mm(pooled[d][0:Kp, CH * jj + t : CH * jj + t + CH]),
                                    start=(t == 0),
                                    stop=(t == 4),
                                )
                            cps = None
                            if corr == "L":
                                # rhs col0 = zero pad col, col1 = pooled col 0
                                cps = cps_pool.tile([128, 2], f32, name="cpsb", tag="cps")
                                nc.tensor.matmul(
                                    cps[0:Mo], lhsT=mm(wlb_t[0:Kp, 0:Mo]),
                                    rhs=mm(pooled[d][0:Kp, 1:3]), start=True, stop=True,
                                )
                            elif corr == "R":
                                # rhs col0 = pooled col WH-1, col1 = zero pad col
                                cps = cps_pool.tile([128, 2], f32, name="cpsb", tag="cps")
                                nc.tensor.matmul(
                                    cps[0:Mo], lhsT=mm(wrb_t[0:Kp, 0:Mo]),
                                    rhs=mm(pooled[d][0:Kp, WH + 1 : WH + 3]), start=True, stop=True,
                                )
                            nc.scalar.copy(
                                out=osb[0:Mo, CH * jh : CH * (jh + 1)], in_=bps[0:Mo]
                            )
                            if corr == "L":
                                nc.vector.tensor_add(
                                    out=osb[0:Mo, 0:1], in0=osb[0:Mo, 0:1],
                                    in1=cps[0:Mo, 1:2],
                                )
                            elif corr == "R":
                                nc.vector.tensor_add(
                                    out=osb[0:Mo, WH // 2 - 1 : WH // 2],
                                    in0=osb[0:Mo, WH // 2 - 1 : WH // 2],
                                    in1=cps[0:Mo, 0:1],
                                )
                        nc.sync.dma_start(
                            out=outdram[BM * d : BM * d + Mo,
                                        (WH // 2) * half : (WH // 2) * (half + 1)],
                            in_=osb[0:Mo],
                        )

    nc.compile()
    return nc


def get_program(loop=False):
    key = "nc_loop" if loop else "nc"
    if key not in _PROGRAM:
        _PROGRAM[key] = _build_program(loop=loop)
    return _PROGRAM[key]


def build_in_maps(x2, w1, w2, w3, w4):
    in_maps = []
    hdt = _host_dt()
    for c in range(NCORES):
        m = {"x": _local_x(x2, c), "zpad": np.zeros((128, 4), hdt)}
        for k, v in _core_weights(c, w1, w2, w3, w4).items():
            m[k] = np.ascontiguousarray(v.astype(hdt))
        in_maps.append(m)
    return in_maps


def kernel(x, w1, w2, w3, w4, H=None, W=None, nTh=None, nTw=None, **_):
    from concourse.bass_utils import run_bass_kernel_spmd

    x2 = np.asarray(x, dtype=np.float32).reshape(8192, 8192)
    ws = [np.asarray(w, dtype=np.float32).reshape(3, 3) for w in (w1, w2, w3, w4)]
    nc = get_program()
    in_maps = build_in_maps(x2, *ws)
    res = run_bass_kernel_spmd(nc, in_maps, core_ids=list(range(NCORES)))
    out = np.concatenate([res.results[c]["out"] for c in range(NCORES)], axis=0)
    return out.reshape(1, 1, 4096, 4096).astype(np.float32)



# revision 38
# speedup vs baseline: 10.1679x; 10.1679x over previous
"""Trainium2 Bass kernel for: conv3x3 -> conv3x3 -> maxpool2x2 -> conv3x3 -> conv3x3
on a [1,1,8192,8192] fp32 image, SAME padding, single channel.

Strategy (8 NeuronCores, height-sharded, halo replicated on host — no collectives):
  * conv1*conv2 are composed into one 5x5 correlation ("stage A"); likewise
    conv3*conv4 ("stage B"). Each 5x5 is computed as 5 PSUM-accumulated
    TensorE band matmuls: the stationary [K<=128, M<=124] band matrix carries
    the 5 vertical taps (mapping input rows on partitions -> output rows),
    and the 5 horizontal taps come from shifting the moving operand's column
    window by t=0..4.
  * Fusing two SAME convs is NOT a plain 5x5 at the image border (the
    reference zeroes the intermediate ring). All corrections are folded into
    the band-matrix *data*: edge-row edits in the main bands, plus per-block
    single-column correction matmuls (bandL/bandR) for the left/right image
    columns, with corner fix-ups. Per-core variants also zero the phantom
    pooled halo rows. The SPMD program is identical on all 8 cores; only the
    band-matrix values differ per core.
  * maxpool2x2: stage-A bands write even/odd output rows to separate
    partition groups, so the row-pair max is a plain partition-sliced
    tensor_tensor max; the column-pair max uses stride-2 access patterns.
    Pooled rows are assembled directly into SBUF-resident stage-B input
    tiles; stage B never touches HBM for its input.
"""

import numpy as np

try:
    import concourse.bass  # noqa: F401
except ImportError:
    import sys
    sys.path.insert(0, "/opt/trn_rl_repo")

H = 8192
W = 8192
NCORES = 8
RPC = H // NCORES          # x rows per core
OPC = RPC // 2             # output rows per core
NZ = RPC + 8               # stage-A output rows computed per core
BM = 124                   # output rows per band-matmul block
NBLK = (NZ + BM - 1) // BM           # 9 stage-A blocks
NBB = (OPC + BM - 1) // BM           # 5 stage-B blocks
WH = W // 2
NSTRIPE = 1
SW = W // NSTRIPE          # output cols per stage-A stripe
CH = 512                   # psum chunk width
NCHUNK_A = SW // CH        # 8
NCHUNK_B = WH // CH        # 8
XROWS = RPC + 16
XCOLS = W + 4
DT_F32 = None  # filled lazily (mybir.dt.float32)


# ------------------------------------------------------------------ bands ---

def _conv_full2d(a, b):
    na, ma = a.shape
    nb, mb = b.shape
    out = np.zeros((na + nb - 1, ma + mb - 1), dtype=np.float64)
    for i in range(na):
        for j in range(ma):
            out[i : i + nb, j : j + mb] += a[i, j] * b
    return out


def _stage_consts(w1, w2):
    w1 = np.asarray(w1, np.float64)
    w2 = np.asarray(w2, np.float64)
    return dict(
        K5=_conv_full2d(w1, w2),
        kh0=np.convolve(w2[0, :], w1[2, :]),
        khb=np.convolve(w2[2, :], w1[0, :]),
        kv0=np.convolve(w2[:, 0], w1[:, 2]),
        kvW=np.convolve(w2[:, 2], w1[:, 0]),
        c00=w2[0, 0] * w1[2, 2],
        c0W=w2[0, 2] * w1[2, 0],
        cH0=w2[2, 0] * w1[0, 2],
        cHW=w2[2, 2] * w1[0, 0],
    )


def _rowmap_permuted(M):
    h = M // 2
    return np.array([2 * m if m < h else 2 * (m - h) + 1 for m in range(M)])


def _build_stage_bands(C, K, M, rowmap, glob_rows, Hout, zero_rows=()):
    """bands [5][K, M], bandL [K, M], bandR [K, M] (float64)."""
    bands = np.zeros((5, K, M), dtype=np.float64)
    bandL = np.zeros((K, M), dtype=np.float64)
    bandR = np.zeros((K, M), dtype=np.float64)
    for m in range(M):
        r = rowmap[m]
        for a in range(5):
            k = r + a
            if k >= K:
                continue
            bands[:, k, m] = C["K5"][a, :]
            bandL[k, m] = -C["kv0"][a]
            bandR[k, m] = -C["kvW"][a]
        g = glob_rows[m]
        k2 = r + 2
        if k2 < K:
            if g == 0:
                bands[:, k2, m] -= C["kh0"]
                bandL[k2, m] += C["c00"]
                bandR[k2, m] += C["c0W"]
            if g == Hout - 1:
                bands[:, k2, m] -= C["khb"]
                bandL[k2, m] += C["cH0"]
                bandR[k2, m] += C["cHW"]
    for k in zero_rows:
        bands[:, k, :] = 0.0
        bandL[k, :] = 0.0
        bandR[k, :] = 0.0
    return bands, bandL, bandR


def _pack(bands):
    """[5, K, M] -> [K, 5*M] matching lhsT slices [K, t*M:(t+1)*M]."""
    b, K, M = bands.shape
    return np.ascontiguousarray(
        bands.transpose(1, 0, 2).reshape(K, b * M).astype(np.float32)
    )


def _scatter_m128(bands, bandL, bandR, Mz):
    """Place dense output cols into the 128-wide layout: even rows at
    partitions 0..h-1, odd rows at 64..64+h-1 (rest zero)."""
    h = Mz // 2
    K = bands.shape[1]
    b128 = np.zeros((5, K, 128), dtype=np.float64)
    l128 = np.zeros((K, 128), dtype=np.float64)
    r128 = np.zeros((K, 128), dtype=np.float64)
    for m in range(Mz):
        col = m if m < h else 64 + (m - h)
        b128[:, :, col] = bands[:, :, m]
        l128[:, col] = bandL[:, m]
        r128[:, col] = bandR[:, m]
    return b128, l128, r128


def _core_weights(c, w1, w2, w3, w4):
    """All weight arrays for core c, keyed by DRAM tensor name."""
    CA = _stage_consts(w1, w2)
    CB = _stage_consts(w3, w4)
    out = {}
    far = np.full(BM, 10 ** 9)  # glob rows that trigger no edits

    # --- stage A ---
    def bands_a(blk):
        Mz = min(BM, NZ - BM * blk)
        Kx = Mz + 4
        rm = _rowmap_permuted(Mz)
        glob = np.array([RPC * c - 4 + BM * blk + rm[m] for m in range(Mz)])
        return _scatter_m128(*_build_stage_bands(CA, Kx, Mz, rm, glob, H), Mz)

    gen = _scatter_m128(
        *_build_stage_bands(CA, 128, BM, _rowmap_permuted(BM), far, H), BM
    )
    blk0 = bands_a(0)
    blk8 = bands_a(NBLK - 1)
    out["wa"], out["wla"], out["wra"] = _pack(gen[0]), *(
        np.ascontiguousarray(g.astype(np.float32)) for g in gen[1:]
    )
    out["wa0"], out["wla0"], out["wra0"] = _pack(blk0[0]), *(
        np.ascontiguousarray(g.astype(np.float32)) for g in blk0[1:]
    )
    out["wa8"], out["wla8"], out["wra8"] = _pack(blk8[0]), *(
        np.ascontiguousarray(g.astype(np.float32)) for g in blk8[1:]
    )

    # --- stage B ---
    def bands_b(d):
        Mo = min(BM, OPC - BM * d)
        Kp = Mo + 4
        rm = np.arange(Mo)
        glob = np.array([OPC * c + BM * d + m for m in range(Mo)])
        zero = [
            k
            for k in range(Kp)
            if not (0 <= OPC * c - 2 + BM * d + k < H // 2)
        ]
        return _build_stage_bands(CB, Kp, Mo, rm, glob, H // 2, zero)

    genb = _build_stage_bands(CB, 128, BM, np.arange(BM), far, H // 2)
    d0 = bands_b(0)
    d4 = bands_b(NBB - 1)
    out["wb"], out["wlb"], out["wrb"] = _pack(genb[0]), *(
        np.ascontiguousarray(g.astype(np.float32)) for g in genb[1:]
    )
    out["wb0"], out["wlb0"], out["wrb0"] = _pack(d0[0]), *(
        np.ascontiguousarray(g.astype(np.float32)) for g in d0[1:]
    )
    out["wb4"], out["wlb4"], out["wrb4"] = _pack(d4[0]), *(
        np.ascontiguousarray(g.astype(np.float32)) for g in d4[1:]
    )
    return out


def _local_x(x2, c):
    """[XROWS, XCOLS] zero-padded slice for core c (rows RPC*c-6 .., cols -2..)."""
    r0 = RPC * c - 6
    out = np.zeros((XROWS, XCOLS), dtype=_host_dt())
    rlo, rhi = max(r0, 0), min(r0 + XROWS, H)
    out[rlo - r0 : rhi - r0, 2 : 2 + W] = x2[rlo:rhi].astype(_host_dt())
    return out


# ---------------------------------------------------------------- program ---

_PROGRAM = {}


MM_DTYPE = "bfloat16"  # "float32" (4 cyc/row, exact), "float32r" (1 cyc/row,
#                          ~2.5e-4 rel err), or "bfloat16" (1 cyc/row, ~1e-3)


def _host_dt():
    if MM_DTYPE == "bfloat16":
        import ml_dtypes
        return ml_dtypes.bfloat16
    return np.float32


def _build_program(loop=False, variant="full", unroll=1):
    # variant: "full" | "mm_only" (PE stream, no drains/pool/stageB)
    #        | "a_only" (stage A incl. pool+scatter) | "dma_only"
    # unroll: bodies emitted per loop iteration (cross-body overlap, no
    #         all-engine barrier between unrolled bodies)
    import concourse.bacc as bacc
    import concourse.mybir as mybir
    import concourse.tile as tile
    from contextlib import ExitStack

    f32 = mybir.dt.float32
    mmdt = getattr(mybir.dt, MM_DTYPE)

    def mm(ap):
        return ap

    # loop: False = straight-line; True = dynamic niter via values_load;
    #       int N = static hardware loop of N iterations (for timing).
    nc = bacc.Bacc("TRN2")

    xdram = nc.dram_tensor("x", [XROWS, XCOLS], mmdt, kind="ExternalInput")
    wshape = {
        "wa": [128, 5 * 128], "wa0": [128, 5 * 128], "wa8": [44, 5 * 128],
        "wb": [128, 5 * BM], "wb0": [128, 5 * BM], "wb4": [20, 5 * 16],
        "wlb": [128, BM], "wrb": [128, BM],
        "wlb0": [128, BM], "wrb0": [128, BM],
        "wlb4": [20, 16], "wrb4": [20, 16],
        # host-precomputed stage-A L/R border corrections, one col per block
        "cal": [128, NBLK], "car": [128, NBLK],
    }
    wdram = {
        k: nc.dram_tensor(k, v, mmdt, kind="ExternalInput")
        for k, v in wshape.items()
    }
    zdram = nc.dram_tensor("zpad", [128, 4], mmdt, kind="ExternalInput")
    if loop is True:
        ndram = nc.dram_tensor("niter", [1, 1], mybir.dt.int32,
                               kind="ExternalInput")
    outdram = nc.dram_tensor("out", [OPC, WH], f32, kind="ExternalOutput")

    with tile.TileContext(nc) as tc:
        with (
            tc.tile_pool(name="wpool", bufs=1) as wpool,
            tc.tile_pool(name="xpool", bufs=3) as xpool,
            tc.tile_pool(name="zpool", bufs=2) as zpool,
            tc.tile_pool(name="hpool", bufs=2) as hpool,
            tc.tile_pool(name="h2pool", bufs=2) as h2pool,
            tc.tile_pool(name="spool", bufs=2) as spool,
            tc.tile_pool(name="ppool", bufs=1) as ppool,
            tc.tile_pool(name="opool", bufs=2) as opool,
            tc.tile_pool(name="zps_pool",
                         bufs=(6 if variant == "mm_outer" else 5),
                         space="PSUM") as zps_pool,
            tc.tile_pool(name="bps_pool",
                         bufs=(1 if variant == "mm_outer" else 2),
                         space="PSUM") as bps_pool,
            tc.tile_pool(name="cps_pool", bufs=1, space="PSUM") as cps_pool,
        ):
            wt = {}
            for k, shp in wshape.items():
                wt[k] = wpool.tile(shp, mmdt, name=f"wt_{k}")
                nc.sync.dma_start(out=wt[k], in_=wdram[k][:])

            if loop is True:
                import concourse.mybir as _mb
                nt = wpool.tile([1, 1], _mb.dt.int32, name="nt")
                nc.sync.dma_start(out=nt, in_=ndram[:])
                nval = nc.values_load(nt[0:1, 0:1], min_val=1, max_val=4096)
                loop_cm = tc.For_i(0, nval, 1, name="rep")
            elif isinstance(loop, int) and loop > 1:
                loop_cm = tc.For_i(0, loop, 1, name="rep")
            else:
                loop_cm = None

            # pad columns never change across iterations: init outside the loop
            pooled = []
            for d in range(NBB):
                pt = ppool.tile([128, WH + 4], mmdt, name=f"pooled{d}")
                pooled.append(pt)
                nc.sync.dma_start(out=pt[:, 0:2], in_=zdram[:, 0:2])
                nc.sync.dma_start(out=pt[:, WH + 2 : WH + 4], in_=zdram[:, 2:4])

            xts = {}

            def load_x(b):
                if b in xts or b >= NBLK:
                    return
                Kx = min(BM, NZ - BM * b) + 4
                xt = xpool.tile([128, W + 4], mmdt, name="xt")
                nc.sync.dma_start(
                    out=xt[0:Kx], in_=xdram[BM * b : BM * b + Kx, :]
                )
                xts[b] = xt

            def emit_stage_a_block(b):
                Mz = min(BM, NZ - BM * b)
                h = Mz // 2
                Kx = Mz + 4
                wa_t = wt["wa0"] if b == 0 else (wt["wa8"] if b == NBLK - 1 else wt["wa"])
                d_t, po = divmod(62 * b, BM)
                xt = xts.pop(b)
                if variant == "mm_outer":
                    # tap-outer: consecutive matmuls share the stationary band
                    for lo, hi in ((0, 6), (6, 12), (12, 16)):
                        zpss = [zps_pool.tile([128, CH], f32, name="zps")
                                for _ in range(hi - lo)]
                        for t in range(5):
                            for jh, jj in enumerate(range(lo, hi)):
                                nc.tensor.matmul(
                                    zpss[jh],
                                    lhsT=mm(wa_t[0:Kx, t * 128 : (t + 1) * 128]),
                                    rhs=mm(xt[0:Kx, CH * jj + t : CH * jj + t + CH]),
                                    start=(t == 0), stop=(t == 4),
                                    skip_group_check=True,
                                )
                    return
                if variant == "mm_1024":
                    for jj in range(NCHUNK_A // 2):
                        zps = zps_pool.tile([128, 2 * CH], f32, name="zps2")
                        for t in range(5):
                            nc.tensor.matmul(
                                zps,
                                lhsT=mm(wa_t[0:Kx, t * 128 : (t + 1) * 128]),
                                rhs=mm(xt[0:Kx, 2 * CH * jj + t : 2 * CH * jj + t + 2 * CH]),
                                start=(t == 0), stop=(t == 4),
                            )
                    return
                zsb = zpool.tile([128, W], mmdt, name="zsb")
                for jj in range(NCHUNK_A):
                    zps = zps_pool.tile([128, CH], f32, name="zps")
                    corr = ("L" if jj == 0 else
                            "R" if jj == NCHUNK_A - 1 else None)
                    if variant == "dma_only":
                        continue
                    for t in range(5):
                        nc.tensor.matmul(
                            zps,
                            lhsT=mm(wa_t[0:Kx, t * 128 : (t + 1) * 128]),
                            rhs=mm(xt[0:Kx, CH * jj + t : CH * jj + t + CH]),
                            start=(t == 0),
                            stop=(t == 4),
                        )
                    if variant == "mm_only":
                        continue
                    c0 = CH * jj
                    nc.scalar.copy(out=zsb[:, c0 : c0 + CH], in_=zps)
                    if corr == "L":
                        nc.vector.tensor_add(
                            out=zsb[:, 0:1], in0=zsb[:, 0:1],
                            in1=wt["cal"][:, b : b + 1],
                        )
                    elif corr == "R":
                        nc.vector.tensor_add(
                            out=zsb[:, W - 1 : W], in0=zsb[:, W - 1 : W],
                            in1=wt["car"][:, b : b + 1],
                        )
                if variant == "mm_only":
                    return
                hp = hpool.tile([128, WH], mmdt, name="hp")
                if variant == "dma_only":
                    st = spool.tile([64, WH], mmdt, name="stg")
                    nc.sync.dma_start(
                        out=pooled[d_t][po : po + h, 2 : 2 + WH], in_=st[0:h]
                    )
                    return
                nc.vector.tensor_max(out=hp, in0=zsb[:, 0:W:2], in1=zsb[:, 1:W:2])
                # row-pair max: TT needs equal base partitions, so first move
                # the odd-row half down to base 0
                hp2 = h2pool.tile([64, WH], mmdt, name="hp2")
                nc.vector.tensor_copy(out=hp2, in_=hp[64:128])
                st = spool.tile([64, WH], mmdt, name="stg")
                nc.vector.tensor_max(out=st[0:h], in0=hp[0:h], in1=hp2[0:h])
                # scatter the block's pooled rows into the stage-B input
                # tiles (DMA: compute engines can't shift partitions)
                nc.sync.dma_start(
                    out=pooled[d_t][po : po + h, 2 : 2 + WH], in_=st[0:h]
                )
                if b >= 2 and b % 2 == 0:
                    # pooled tiles overlap by 4 rows at band boundaries
                    nc.sync.dma_start(
                        out=pooled[b // 2 - 1][BM : BM + 4, 2 : 2 + WH],
                        in_=st[0:4],
                    )

            def emit_stage_b_block(d):
                Mo = min(BM, OPC - BM * d)
                Kp = Mo + 4
                wb_t = wt["wb0"] if d == 0 else (wt["wb4"] if d == NBB - 1 else wt["wb"])
                wlb_t = wt["wlb0"] if d == 0 else (wt["wlb4"] if d == NBB - 1 else wt["wlb"])
                wrb_t = wt["wrb0"] if d == 0 else (wt["wrb4"] if d == NBB - 1 else wt["wrb"])
                for half in range(2):
                    osb = opool.tile([BM, WH // 2], f32, name="osb")
                    for jh in range(NCHUNK_B // 2):
                        jj = half * (NCHUNK_B // 2) + jh
                        bps = bps_pool.tile([BM, CH], f32, name="bps")
                        corr = ("L" if jj == 0 else
                                "R" if jj == NCHUNK_B - 1 else None)
                        if variant == "dma_only":
                            continue
                        for t in range(5):
                            nc.tensor.matmul(
                                bps[0:Mo],
                                lhsT=mm(wb_t[0:Kp, t * Mo : (t + 1) * Mo]),
                                rhs=mm(pooled[d][0:Kp, CH * jj + t : CH * jj + t + CH]),
                                start=(t == 0),
                                stop=(t == 4),
                            )
                        if variant in ("mm_only", "mm_outer", "mm_1024"):
                            continue
                        cps = None
                        if corr == "L":
                            # rhs col0 = zero pad col, col1 = pooled col 0
                            cps = cps_pool.tile([128, 2], f32, name="cpsb", tag="cps")
                            nc.tensor.matmul(
                                cps[0:Mo], lhsT=mm(wlb_t[0:Kp, 0:Mo]),
                                rhs=mm(pooled[d][0:Kp, 1:3]), start=True, stop=True,
                            )
                        elif corr == "R":
                            # rhs col0 = pooled col WH-1, col1 = zero pad col
                            cps = cps_pool.tile([128, 2], f32, name="cpsb", tag="cps")
                            nc.tensor.matmul(
                                cps[0:Mo], lhsT=mm(wrb_t[0:Kp, 0:Mo]),
                                rhs=mm(pooled[d][0:Kp, WH + 1 : WH + 3]), start=True, stop=True,
                            )
                        nc.scalar.copy(
                            out=osb[0:Mo, CH * jh : CH * (jh + 1)], in_=bps[0:Mo]
                        )
                        if corr == "L":
                            nc.vector.tensor_add(
                                out=osb[0:Mo, 0:1], in0=osb[0:Mo, 0:1],
                                in1=cps[0:Mo, 1:2],
                            )
                        elif corr == "R":
                            nc.vector.tensor_add(
                                out=osb[0:Mo, WH // 2 - 1 : WH // 2],
                                in0=osb[0:Mo, WH // 2 - 1 : WH // 2],
                                in1=cps[0:Mo, 0:1],
                            )
                    if variant not in ("mm_only", "mm_outer", "mm_1024"):
                        nc.sync.dma_start(
                            out=outdram[BM * d : BM * d + Mo,
                                        (WH // 2) * half : (WH // 2) * (half + 1)],
                            in_=osb[0:Mo],
                        )

            with (loop_cm if loop_cm is not None else ExitStack()):
                # Interleave: B-block d only needs A-blocks <= 2d+2, so run it
                # as soon as its pooled tile is complete. B's ACT/DMA work
                # then overlaps A's PE-bound stream instead of serializing.
                # B-block d is ready after A-block 2d+2; hold B2 back so it
                # covers A8's drain (B3/B4 are forced to the tail anyway).
                do_b = variant != "a_only"
                for _u in range(unroll):
                    load_x(0)
                    for b in range(NBLK):
                        load_x(b + 1)
                        emit_stage_a_block(b)
                        if do_b and b in (2, 4):
                            emit_stage_b_block(b // 2 - 1)
                    if do_b:
                        for d in (2, 3, 4):
                            emit_stage_b_block(d)

    nc.compile()
    return nc


def get_program(loop=False, variant="full", unroll=1):
    key = f"nc_{loop}_{variant}_{unroll}"
    if key not in _PROGRAM:
        _PROGRAM[key] = _build_program(loop=loop, variant=variant, unroll=unroll)
    return _PROGRAM[key]


def build_in_maps(x2, w1, w2, w3, w4):
    in_maps = []
    hdt = _host_dt()
    for c in range(NCORES):
        m = {"x": _local_x(x2, c), "zpad": np.zeros((128, 4), hdt)}
        cw = _core_weights(c, w1, w2, w3, w4)
        # stage-A L/R border corrections, evaluated on host (f64) instead of
        # on-device single-column matmuls
        r0 = RPC * c - 6
        xcol = np.zeros((XROWS, 2), np.float64)
        rlo, rhi = max(r0, 0), min(r0 + XROWS, H)
        xcol[rlo - r0 : rhi - r0, 0] = x2[rlo:rhi, 0]
        xcol[rlo - r0 : rhi - r0, 1] = x2[rlo:rhi, W - 1]
        cal = np.zeros((128, NBLK), np.float64)
        car = np.zeros((128, NBLK), np.float64)
        for b in range(NBLK):
            Kx = min(BM, NZ - BM * b) + 4
            wl = cw["wla0"] if b == 0 else (cw["wla8"] if b == NBLK - 1 else cw["wla"])
            wr = cw["wra0"] if b == 0 else (cw["wra8"] if b == NBLK - 1 else cw["wra"])
            cal[:, b] = wl.astype(np.float64).T @ xcol[BM * b : BM * b + Kx, 0]
            car[:, b] = wr.astype(np.float64).T @ xcol[BM * b : BM * b + Kx, 1]
        m["cal"] = np.ascontiguousarray(cal.astype(hdt))
        m["car"] = np.ascontiguousarray(car.astype(hdt))
        for k, v in cw.items():
            if k.startswith("wla") or k.startswith("wra"):
                continue
            m[k] = np.ascontiguousarray(v.astype(hdt))
        in_maps.append(m)
    return in_maps


def kernel(x, w1, w2, w3, w4, H=None, W=None, nTh=None, nTw=None, **_):
    from concourse.bass_utils import run_bass_kernel_spmd

    x2 = np.asarray(x, dtype=np.float32).reshape(8192, 8192)
    ws = [np.asarray(w, dtype=np.float32).reshape(3, 3) for w in (w1, w2, w3, w4)]
    nc = get_program()
    in_maps = build_in_maps(x2, *ws)
    res = run_bass_kernel_spmd(nc, in_maps, core_ids=list(range(NCORES)))
    out = np.concatenate([res.results[c]["out"] for c in range(NCORES)], axis=0)
    return out.reshape(1, 1, 4096, 4096).astype(np.float32)



# revision 41
# speedup vs baseline: 11.4788x; 1.1289x over previous
"""Trainium2 Bass kernel for: conv3x3 -> conv3x3 -> maxpool2x2 -> conv3x3 -> conv3x3
on a [1,1,8192,8192] fp32 image, SAME padding, single channel.

Strategy (8 NeuronCores, height-sharded, halo replicated on host — no collectives):
  * conv1*conv2 are composed into one 5x5 correlation ("stage A"); likewise
    conv3*conv4 ("stage B"). Each 5x5 is computed as 5 PSUM-accumulated
    TensorE band matmuls: the stationary [K<=128, M<=124] band matrix carries
    the 5 vertical taps (mapping input rows on partitions -> output rows),
    and the 5 horizontal taps come from shifting the moving operand's column
    window by t=0..4.
  * Fusing two SAME convs is NOT a plain 5x5 at the image border (the
    reference zeroes the intermediate ring). All corrections are folded into
    the band-matrix *data*: edge-row edits in the main bands, plus per-block
    single-column correction matmuls (bandL/bandR) for the left/right image
    columns, with corner fix-ups. Per-core variants also zero the phantom
    pooled halo rows. The SPMD program is identical on all 8 cores; only the
    band-matrix values differ per core.
  * maxpool2x2: stage-A bands write even/odd output rows to separate
    partition groups, so the row-pair max is a plain partition-sliced
    tensor_tensor max; the column-pair max uses stride-2 access patterns.
    Pooled rows are assembled directly into SBUF-resident stage-B input
    tiles; stage B never touches HBM for its input.
"""

import numpy as np

try:
    import concourse.bass  # noqa: F401
except ImportError:
    import sys
    sys.path.insert(0, "/opt/trn_rl_repo")

H = 8192
W = 8192
NCORES = 8
RPC = H // NCORES          # x rows per core
OPC = RPC // 2             # output rows per core
NZ = RPC + 8               # stage-A output rows computed per core
BM = 124                   # output rows per band-matmul block
NBLK = (NZ + BM - 1) // BM           # 9 stage-A blocks
NBB = (OPC + BM - 1) // BM           # 5 stage-B blocks
WH = W // 2
NSTRIPE = 1
SW = W // NSTRIPE          # output cols per stage-A stripe
CH = 512                   # psum chunk width
NCHUNK_A = SW // CH        # 8
NCHUNK_B = WH // CH        # 8
XROWS = RPC + 16
XCOLS = W + 4
DT_F32 = None  # filled lazily (mybir.dt.float32)


# ------------------------------------------------------------------ bands ---

def _conv_full2d(a, b):
    na, ma = a.shape
    nb, mb = b.shape
    out = np.zeros((na + nb - 1, ma + mb - 1), dtype=np.float64)
    for i in range(na):
        for j in range(ma):
            out[i : i + nb, j : j + mb] += a[i, j] * b
    return out


def _stage_consts(w1, w2):
    w1 = np.asarray(w1, np.float64)
    w2 = np.asarray(w2, np.float64)
    return dict(
        K5=_conv_full2d(w1, w2),
        kh0=np.convolve(w2[0, :], w1[2, :]),
        khb=np.convolve(w2[2, :], w1[0, :]),
        kv0=np.convolve(w2[:, 0], w1[:, 2]),
        kvW=np.convolve(w2[:, 2], w1[:, 0]),
        c00=w2[0, 0] * w1[2, 2],
        c0W=w2[0, 2] * w1[2, 0],
        cH0=w2[2, 0] * w1[0, 2],
        cHW=w2[2, 2] * w1[0, 0],
    )


def _rowmap_permuted(M):
    h = M // 2
    return np.array([2 * m if m < h else 2 * (m - h) + 1 for m in range(M)])


def _build_stage_bands(C, K, M, rowmap, glob_rows, Hout, zero_rows=()):
    """bands [5][K, M], bandL [K, M], bandR [K, M] (float64)."""
    bands = np.zeros((5, K, M), dtype=np.float64)
    bandL = np.zeros((K, M), dtype=np.float64)
    bandR = np.zeros((K, M), dtype=np.float64)
    for m in range(M):
        r = rowmap[m]
        for a in range(5):
            k = r + a
            if k >= K:
                continue
            bands[:, k, m] = C["K5"][a, :]
            bandL[k, m] = -C["kv0"][a]
            bandR[k, m] = -C["kvW"][a]
        g = glob_rows[m]
        k2 = r + 2
        if k2 < K:
            if g == 0:
                bands[:, k2, m] -= C["kh0"]
                bandL[k2, m] += C["c00"]
                bandR[k2, m] += C["c0W"]
            if g == Hout - 1:
                bands[:, k2, m] -= C["khb"]
                bandL[k2, m] += C["cH0"]
                bandR[k2, m] += C["cHW"]
    for k in zero_rows:
        bands[:, k, :] = 0.0
        bandL[k, :] = 0.0
        bandR[k, :] = 0.0
    return bands, bandL, bandR


def _pack(bands):
    """[5, K, M] -> [K, 5*M] matching lhsT slices [K, t*M:(t+1)*M]."""
    b, K, M = bands.shape
    return np.ascontiguousarray(
        bands.transpose(1, 0, 2).reshape(K, b * M).astype(np.float32)
    )


def _scatter_m128(bands, bandL, bandR, Mz):
    """Place dense output cols into the 128-wide layout: even rows at
    partitions 0..h-1, odd rows at 64..64+h-1 (rest zero)."""
    h = Mz // 2
    K = bands.shape[1]
    b128 = np.zeros((5, K, 128), dtype=np.float64)
    l128 = np.zeros((K, 128), dtype=np.float64)
    r128 = np.zeros((K, 128), dtype=np.float64)
    for m in range(Mz):
        col = m if m < h else 64 + (m - h)
        b128[:, :, col] = bands[:, :, m]
        l128[:, col] = bandL[:, m]
        r128[:, col] = bandR[:, m]
    return b128, l128, r128


def _core_weights(c, w1, w2, w3, w4):
    """All weight arrays for core c, keyed by DRAM tensor name."""
    CA = _stage_consts(w1, w2)
    CB = _stage_consts(w3, w4)
    out = {}
    far = np.full(BM, 10 ** 9)  # glob rows that trigger no edits

    # --- stage A ---
    def bands_a(blk):
        Mz = min(BM, NZ - BM * blk)
        Kx = Mz + 4
        rm = _rowmap_permuted(Mz)
        glob = np.array([RPC * c - 4 + BM * blk + rm[m] for m in range(Mz)])
        return _scatter_m128(*_build_stage_bands(CA, Kx, Mz, rm, glob, H), Mz)

    gen = _scatter_m128(
        *_build_stage_bands(CA, 128, BM, _rowmap_permuted(BM), far, H), BM
    )
    blk0 = bands_a(0)
    blk8 = bands_a(NBLK - 1)
    out["wa"], out["wla"], out["wra"] = _pack(gen[0]), *(
        np.ascontiguousarray(g.astype(np.float32)) for g in gen[1:]
    )
    out["wa0"], out["wla0"], out["wra0"] = _pack(blk0[0]), *(
        np.ascontiguousarray(g.astype(np.float32)) for g in blk0[1:]
    )
    out["wa8"], out["wla8"], out["wra8"] = _pack(blk8[0]), *(
        np.ascontiguousarray(g.astype(np.float32)) for g in blk8[1:]
    )

    # --- stage B ---
    def bands_b(d):
        Mo = min(BM, OPC - BM * d)
        Kp = Mo + 4
        rm = np.arange(Mo)
        glob = np.array([OPC * c + BM * d + m for m in range(Mo)])
        zero = [
            k
            for k in range(Kp)
            if not (0 <= OPC * c - 2 + BM * d + k < H // 2)
        ]
        return _build_stage_bands(CB, Kp, Mo, rm, glob, H // 2, zero)

    genb = _build_stage_bands(CB, 128, BM, np.arange(BM), far, H // 2)
    d0 = bands_b(0)
    d4 = bands_b(NBB - 1)
    out["wb"], out["wlb"], out["wrb"] = _pack(genb[0]), *(
        np.ascontiguousarray(g.astype(np.float32)) for g in genb[1:]
    )
    out["wb0"], out["wlb0"], out["wrb0"] = _pack(d0[0]), *(
        np.ascontiguousarray(g.astype(np.float32)) for g in d0[1:]
    )
    out["wb4"], out["wlb4"], out["wrb4"] = _pack(d4[0]), *(
        np.ascontiguousarray(g.astype(np.float32)) for g in d4[1:]
    )
    return out


def _local_x(x2, c):
    """[XROWS, XCOLS] zero-padded slice for core c (rows RPC*c-6 .., cols -2..)."""
    r0 = RPC * c - 6
    out = np.zeros((XROWS, XCOLS), dtype=_host_dt())
    rlo, rhi = max(r0, 0), min(r0 + XROWS, H)
    out[rlo - r0 : rhi - r0, 2 : 2 + W] = x2[rlo:rhi].astype(_host_dt())
    return out


# ---------------------------------------------------------------- program ---

_PROGRAM = {}


MM_DTYPE = "bfloat16"  # "float32" (4 cyc/row, exact), "float32r" (1 cyc/row,
#                          ~2.5e-4 rel err), or "bfloat16" (1 cyc/row, ~1e-3)


def _host_dt():
    if MM_DTYPE == "bfloat16":
        import ml_dtypes
        return ml_dtypes.bfloat16
    return np.float32


def _build_program(loop=False, variant="full", unroll=1):
    # variant: "full" | "mm_only" (PE stream, no drains/pool/stageB)
    #        | "a_only" (stage A incl. pool+scatter) | "dma_only"
    # unroll: bodies emitted per loop iteration (cross-body overlap, no
    #         all-engine barrier between unrolled bodies)
    import concourse.bacc as bacc
    import concourse.mybir as mybir
    import concourse.tile as tile
    from contextlib import ExitStack

    f32 = mybir.dt.float32
    mmdt = getattr(mybir.dt, MM_DTYPE)

    def mm(ap):
        return ap

    # loop: False = straight-line; True = dynamic niter via values_load;
    #       int N = static hardware loop of N iterations (for timing).
    nc = bacc.Bacc("TRN2")

    xdram = nc.dram_tensor("x", [XROWS, XCOLS], mmdt, kind="ExternalInput")
    wshape = {
        "wa": [128, 5 * 128], "wa0": [128, 5 * 128], "wa8": [44, 5 * 128],
        "wb": [128, 5 * BM], "wb0": [128, 5 * BM], "wb4": [20, 5 * 16],
        "wlb": [128, BM], "wrb": [128, BM],
        "wlb0": [128, BM], "wrb0": [128, BM],
        "wlb4": [20, 16], "wrb4": [20, 16],
        # host-precomputed stage-A L/R border corrections, one col per block
        "cal": [128, NBLK], "car": [128, NBLK],
    }
    wdram = {
        k: nc.dram_tensor(k, v, mmdt, kind="ExternalInput")
        for k, v in wshape.items()
    }
    zdram = nc.dram_tensor("zpad", [128, 4], mmdt, kind="ExternalInput")
    if loop is True:
        ndram = nc.dram_tensor("niter", [1, 1], mybir.dt.int32,
                               kind="ExternalInput")
    outdram = nc.dram_tensor("out", [OPC, WH], mmdt, kind="ExternalOutput")

    with tile.TileContext(nc) as tc:
        with (
            tc.tile_pool(name="wpool", bufs=1) as wpool,
            tc.tile_pool(name="xpool", bufs=3) as xpool,
            tc.tile_pool(name="zpool", bufs=3) as zpool,
            tc.tile_pool(name="hpool", bufs=2) as hpool,
            tc.tile_pool(name="h2pool", bufs=2) as h2pool,
            tc.tile_pool(name="spool", bufs=2) as spool,
            tc.tile_pool(name="ppool", bufs=1) as ppool,
            tc.tile_pool(name="opool", bufs=2) as opool,
            tc.tile_pool(name="zps_pool",
                         bufs=(6 if variant == "mm_outer" else 5),
                         space="PSUM") as zps_pool,
            tc.tile_pool(name="bps_pool",
                         bufs=(1 if variant == "mm_outer" else 2),
                         space="PSUM") as bps_pool,
            tc.tile_pool(name="cps_pool", bufs=1, space="PSUM") as cps_pool,
        ):
            wt = {}
            for k, shp in wshape.items():
                wt[k] = wpool.tile(shp, mmdt, name=f"wt_{k}")
                nc.sync.dma_start(out=wt[k], in_=wdram[k][:])

            if loop is True:
                import concourse.mybir as _mb
                nt = wpool.tile([1, 1], _mb.dt.int32, name="nt")
                nc.sync.dma_start(out=nt, in_=ndram[:])
                nval = nc.values_load(nt[0:1, 0:1], min_val=1, max_val=4096)
                loop_cm = tc.For_i(0, nval, 1, name="rep")
            elif isinstance(loop, int) and loop > 1:
                loop_cm = tc.For_i(0, loop, 1, name="rep")
            else:
                loop_cm = None

            # pad columns never change across iterations: init outside the loop
            pooled = []
            for d in range(NBB):
                pt = ppool.tile([128, WH + 4], mmdt, name=f"pooled{d}")
                pooled.append(pt)
                nc.sync.dma_start(out=pt[:, 0:2], in_=zdram[:, 0:2])
                nc.sync.dma_start(out=pt[:, WH + 2 : WH + 4], in_=zdram[:, 2:4])

            xts = {}

            def load_x(b):
                if b in xts or b >= NBLK:
                    return
                Kx = min(BM, NZ - BM * b) + 4
                xt = xpool.tile([128, W + 4], mmdt, name="xt")
                nc.sync.dma_start(
                    out=xt[0:Kx], in_=xdram[BM * b : BM * b + Kx, :]
                )
                xts[b] = xt

            def emit_stage_a_block(b):
                Mz = min(BM, NZ - BM * b)
                h = Mz // 2
                Kx = Mz + 4
                wa_t = wt["wa0"] if b == 0 else (wt["wa8"] if b == NBLK - 1 else wt["wa"])
                d_t, po = divmod(62 * b, BM)
                xt = xts.pop(b)
                if variant == "mm_outer":
                    # tap-outer: consecutive matmuls share the stationary band
                    for lo, hi in ((0, 6), (6, 12), (12, 16)):
                        zpss = [zps_pool.tile([128, CH], f32, name="zps")
                                for _ in range(hi - lo)]
                        for t in range(5):
                            for jh, jj in enumerate(range(lo, hi)):
                                nc.tensor.matmul(
                                    zpss[jh],
                                    lhsT=mm(wa_t[0:Kx, t * 128 : (t + 1) * 128]),
                                    rhs=mm(xt[0:Kx, CH * jj + t : CH * jj + t + CH]),
                                    start=(t == 0), stop=(t == 4),
                                    skip_group_check=True,
                                )
                    return
                if variant == "mm_1024":
                    for jj in range(NCHUNK_A // 2):
                        zps = zps_pool.tile([128, 2 * CH], f32, name="zps2")
                        for t in range(5):
                            nc.tensor.matmul(
                                zps,
                                lhsT=mm(wa_t[0:Kx, t * 128 : (t + 1) * 128]),
                                rhs=mm(xt[0:Kx, 2 * CH * jj + t : 2 * CH * jj + t + 2 * CH]),
                                start=(t == 0), stop=(t == 4),
                            )
                    return
                zsb = zpool.tile([128, W], mmdt, name="zsb")
                for jj in range(NCHUNK_A):
                    zps = zps_pool.tile([128, CH], f32, name="zps")
                    corr = ("L" if jj == 0 else
                            "R" if jj == NCHUNK_A - 1 else None)
                    if variant == "dma_only":
                        continue
                    for t in range(5):
                        nc.tensor.matmul(
                            zps,
                            lhsT=mm(wa_t[0:Kx, t * 128 : (t + 1) * 128]),
                            rhs=mm(xt[0:Kx, CH * jj + t : CH * jj + t + CH]),
                            start=(t == 0),
                            stop=(t == 4),
                        )
                    if variant == "mm_only":
                        continue
                    c0 = CH * jj
                    nc.scalar.copy(out=zsb[:, c0 : c0 + CH], in_=zps)
                    if corr == "L":
                        nc.vector.tensor_add(
                            out=zsb[:, 0:1], in0=zsb[:, 0:1],
                            in1=wt["cal"][:, b : b + 1],
                        )
                    elif corr == "R":
                        nc.vector.tensor_add(
                            out=zsb[:, W - 1 : W], in0=zsb[:, W - 1 : W],
                            in1=wt["car"][:, b : b + 1],
                        )
                if variant == "mm_only":
                    return
                hp = hpool.tile([128, WH], mmdt, name="hp")
                if variant == "dma_only":
                    st = spool.tile([64, WH], mmdt, name="stg")
                    nc.sync.dma_start(
                        out=pooled[d_t][po : po + h, 2 : 2 + WH], in_=st[0:h]
                    )
                    return
                nc.vector.tensor_max(out=hp, in0=zsb[:, 0:W:2], in1=zsb[:, 1:W:2])
                # row-pair max: TT needs equal base partitions, so first move
                # the odd-row half down to base 0
                hp2 = h2pool.tile([64, WH], mmdt, name="hp2")
                nc.vector.tensor_copy(out=hp2, in_=hp[64:128])
                st = spool.tile([64, WH], mmdt, name="stg")
                nc.vector.tensor_max(out=st[0:h], in0=hp[0:h], in1=hp2[0:h])
                # scatter the block's pooled rows into the stage-B input
                # tiles (DMA: compute engines can't shift partitions)
                nc.sync.dma_start(
                    out=pooled[d_t][po : po + h, 2 : 2 + WH], in_=st[0:h]
                )
                if b >= 2 and b % 2 == 0:
                    # pooled tiles overlap by 4 rows at band boundaries
                    nc.sync.dma_start(
                        out=pooled[b // 2 - 1][BM : BM + 4, 2 : 2 + WH],
                        in_=st[0:4],
                    )

            def emit_stage_b_block(d):
                Mo = min(BM, OPC - BM * d)
                Kp = Mo + 4
                wb_t = wt["wb0"] if d == 0 else (wt["wb4"] if d == NBB - 1 else wt["wb"])
                wlb_t = wt["wlb0"] if d == 0 else (wt["wlb4"] if d == NBB - 1 else wt["wlb"])
                wrb_t = wt["wrb0"] if d == 0 else (wt["wrb4"] if d == NBB - 1 else wt["wrb"])
                for half in range(2):
                    osb = opool.tile([BM, WH // 2], mmdt, name="osb")
                    for jh in range(NCHUNK_B // 2):
                        jj = half * (NCHUNK_B // 2) + jh
                        bps = bps_pool.tile([BM, CH], f32, name="bps")
                        corr = ("L" if jj == 0 else
                                "R" if jj == NCHUNK_B - 1 else None)
                        if variant == "dma_only":
                            continue
                        for t in range(5):
                            nc.tensor.matmul(
                                bps[0:Mo],
                                lhsT=mm(wb_t[0:Kp, t * Mo : (t + 1) * Mo]),
                                rhs=mm(pooled[d][0:Kp, CH * jj + t : CH * jj + t + CH]),
                                start=(t == 0),
                                stop=(t == 4),
                            )
                        if variant in ("mm_only", "mm_outer", "mm_1024"):
                            continue
                        cps = None
                        if corr == "L":
                            # rhs col0 = zero pad col, col1 = pooled col 0
                            cps = cps_pool.tile([128, 2], f32, name="cpsb", tag="cps")
                            nc.tensor.matmul(
                                cps[0:Mo], lhsT=mm(wlb_t[0:Kp, 0:Mo]),
                                rhs=mm(pooled[d][0:Kp, 1:3]), start=True, stop=True,
                            )
                        elif corr == "R":
                            # rhs col0 = pooled col WH-1, col1 = zero pad col
                            cps = cps_pool.tile([128, 2], f32, name="cpsb", tag="cps")
                            nc.tensor.matmul(
                                cps[0:Mo], lhsT=mm(wrb_t[0:Kp, 0:Mo]),
                                rhs=mm(pooled[d][0:Kp, WH + 1 : WH + 3]), start=True, stop=True,
                            )
                        nc.scalar.copy(
                            out=osb[0:Mo, CH * jh : CH * (jh + 1)], in_=bps[0:Mo]
                        )
                        if corr == "L":
                            nc.vector.tensor_add(
                                out=osb[0:Mo, 0:1], in0=osb[0:Mo, 0:1],
                                in1=cps[0:Mo, 1:2],
                            )
                        elif corr == "R":
                            nc.vector.tensor_add(
                                out=osb[0:Mo, WH // 2 - 1 : WH // 2],
                                in0=osb[0:Mo, WH // 2 - 1 : WH // 2],
                                in1=cps[0:Mo, 0:1],
                            )
                    if variant not in ("mm_only", "mm_outer", "mm_1024"):
                        nc.sync.dma_start(
                            out=outdram[BM * d : BM * d + Mo,
                                        (WH // 2) * half : (WH // 2) * (half + 1)],
                            in_=osb[0:Mo],
                        )

            with (loop_cm if loop_cm is not None else ExitStack()):
                # Interleave: B-block d only needs A-blocks <= 2d+2, so run it
                # as soon as its pooled tile is complete. B's ACT/DMA work
                # then overlaps A's PE-bound stream instead of serializing.
                # B-block d is ready after A-block 2d+2; hold B2 back so it
                # covers A8's drain (B3/B4 are forced to the tail anyway).
                do_b = variant != "a_only"
                for _u in range(unroll):
                    load_x(0)
                    for b in range(NBLK):
                        load_x(b + 1)
                        emit_stage_a_block(b)
                        if do_b and b in (2, 4):
                            emit_stage_b_block(b // 2 - 1)
                    if do_b:
                        for d in (2, 3, 4):
                            emit_stage_b_block(d)

    nc.compile()
    return nc


def get_program(loop=False, variant="full", unroll=1):
    key = f"nc_{loop}_{variant}_{unroll}"
    if key not in _PROGRAM:
        _PROGRAM[key] = _build_program(loop=loop, variant=variant, unroll=unroll)
    return _PROGRAM[key]


def build_in_maps(x2, w1, w2, w3, w4):
    in_maps = []
    hdt = _host_dt()
    for c in range(NCORES):
        m = {"x": _local_x(x2, c), "zpad": np.zeros((128, 4), hdt)}
        cw = _core_weights(c, w1, w2, w3, w4)
        # stage-A L/R border corrections, evaluated on host (f64) instead of
        # on-device single-column matmuls
        r0 = RPC * c - 6
        xcol = np.zeros((XROWS, 2), np.float64)
        rlo, rhi = max(r0, 0), min(r0 + XROWS, H)
        xcol[rlo - r0 : rhi - r0, 0] = x2[rlo:rhi, 0]
        xcol[rlo - r0 : rhi - r0, 1] = x2[rlo:rhi, W - 1]
        cal = np.zeros((128, NBLK), np.float64)
        car = np.zeros((128, NBLK), np.float64)
        for b in range(NBLK):
            Kx = min(BM, NZ - BM * b) + 4
            wl = cw["wla0"] if b == 0 else (cw["wla8"] if b == NBLK - 1 else cw["wla"])
            wr = cw["wra0"] if b == 0 else (cw["wra8"] if b == NBLK - 1 else cw["wra"])
            cal[:, b] = wl.astype(np.float64).T @ xcol[BM * b : BM * b + Kx, 0]
            car[:, b] = wr.astype(np.float64).T @ xcol[BM * b : BM * b + Kx, 1]
        m["cal"] = np.ascontiguousarray(cal.astype(hdt))
        m["car"] = np.ascontiguousarray(car.astype(hdt))
        for k, v in cw.items():
            if k.startswith("wla") or k.startswith("wra"):
                continue
            m[k] = np.ascontiguousarray(v.astype(hdt))
        in_maps.append(m)
    return in_maps


def kernel(x, w1, w2, w3, w4, H=None, W=None, nTh=None, nTw=None, **_):
    from concourse.bass_utils import run_bass_kernel_spmd

    x2 = np.asarray(x, dtype=np.float32).reshape(8192, 8192)
    ws = [np.asarray(w, dtype=np.float32).reshape(3, 3) for w in (w1, w2, w3, w4)]
    nc = get_program()
    in_maps = build_in_maps(x2, *ws)
    res = run_bass_kernel_spmd(nc, in_maps, core_ids=list(range(NCORES)))
    out = np.concatenate([res.results[c]["out"] for c in range(NCORES)], axis=0)
    return out.reshape(1, 1, 4096, 4096).astype(np.float32)



# revision 47
# speedup vs baseline: 12.2231x; 1.0648x over previous
"""Trainium2 Bass kernel for: conv3x3 -> conv3x3 -> maxpool2x2 -> conv3x3 -> conv3x3
on a [1,1,8192,8192] fp32 image, SAME padding, single channel.

Strategy (8 NeuronCores, height-sharded, halo replicated on host — no collectives):
  * conv1*conv2 are composed into one 5x5 correlation ("stage A"); likewise
    conv3*conv4 ("stage B"). Each 5x5 is computed as 5 PSUM-accumulated
    TensorE band matmuls: the stationary [K<=128, M<=124] band matrix carries
    the 5 vertical taps (mapping input rows on partitions -> output rows),
    and the 5 horizontal taps come from shifting the moving operand's column
    window by t=0..4. Everything is bf16 (matmuls, SBUF tiles, DMA) except
    PSUM accumulation, which is always fp32.
  * Fusing two SAME convs is NOT a plain 5x5 at the image border (the
    reference zeroes the intermediate ring). Edge-row edits live in the main
    band data; the left/right image-column corrections for stage A are
    precomputed on the host (they only depend on x's first/last column) and
    added as single-column tensor_adds, while stage B's are per-block
    single-column correction matmuls against the pooled data. Per-core
    variants also zero the phantom pooled halo rows. The SPMD program is
    identical on all 8 cores; only the band/correction values differ.
  * maxpool2x2: stage-A bands write even/odd output rows to separate
    partition groups. Per 124-row block the full-width [128, 8192] PSUM
    output is drained by ACT into one bf16 SBUF tile, then three wide DVE
    ops do the pool (stride-2 column max, odd-half realign copy, row-pair
    max). Pooled rows are scattered by DMA into SBUF-resident stage-B input
    tiles; stage B never touches HBM for its input.
  * Stage-B blocks are interleaved into the stage-A stream as soon as their
    pooled inputs are complete, keeping TensorE (the critical engine, ~100%
    busy) fed through what would otherwise be pool-drain stalls. x tiles are
    prefetched one block ahead.
  * _build_program(loop=N, unroll=U) wraps the body in a static hardware
    For_i loop with U bodies per iteration — used by test.py to measure the
    true per-execution device time as a slope, free of the ~0.7ms host/axon
    dispatch overhead that dominates per-call wall time.
"""

import numpy as np

try:
    import concourse.bass  # noqa: F401
except ImportError:
    import sys
    sys.path.insert(0, "/opt/trn_rl_repo")

H = 8192
W = 8192
NCORES = 8
RPC = H // NCORES          # x rows per core
OPC = RPC // 2             # output rows per core
NZ = RPC + 8               # stage-A output rows computed per core
BM = 124                   # output rows per band-matmul block
NBLK = (NZ + BM - 1) // BM           # 9 stage-A blocks
NBB = (OPC + BM - 1) // BM           # 5 stage-B blocks
WH = W // 2
NSTRIPE = 1
SW = W // NSTRIPE          # output cols per stage-A stripe
CH = 512                   # psum chunk width
NCHUNK_A = SW // CH        # 8
NCHUNK_B = WH // CH        # 8
XROWS = RPC + 16
XCOLS = W + 4
DT_F32 = None  # filled lazily (mybir.dt.float32)


# ------------------------------------------------------------------ bands ---

def _conv_full2d(a, b):
    na, ma = a.shape
    nb, mb = b.shape
    out = np.zeros((na + nb - 1, ma + mb - 1), dtype=np.float64)
    for i in range(na):
        for j in range(ma):
            out[i : i + nb, j : j + mb] += a[i, j] * b
    return out


def _stage_consts(w1, w2):
    w1 = np.asarray(w1, np.float64)
    w2 = np.asarray(w2, np.float64)
    return dict(
        K5=_conv_full2d(w1, w2),
        kh0=np.convolve(w2[0, :], w1[2, :]),
        khb=np.convolve(w2[2, :], w1[0, :]),
        kv0=np.convolve(w2[:, 0], w1[:, 2]),
        kvW=np.convolve(w2[:, 2], w1[:, 0]),
        c00=w2[0, 0] * w1[2, 2],
        c0W=w2[0, 2] * w1[2, 0],
        cH0=w2[2, 0] * w1[0, 2],
        cHW=w2[2, 2] * w1[0, 0],
    )


def _rowmap_permuted(M):
    h = M // 2
    return np.array([2 * m if m < h else 2 * (m - h) + 1 for m in range(M)])


def _build_stage_bands(C, K, M, rowmap, glob_rows, Hout, zero_rows=()):
    """bands [5][K, M], bandL [K, M], bandR [K, M] (float64)."""
    bands = np.zeros((5, K, M), dtype=np.float64)
    bandL = np.zeros((K, M), dtype=np.float64)
    bandR = np.zeros((K, M), dtype=np.float64)
    for m in range(M):
        r = rowmap[m]
        for a in range(5):
            k = r + a
            if k >= K:
                continue
            bands[:, k, m] = C["K5"][a, :]
            bandL[k, m] = -C["kv0"][a]
            bandR[k, m] = -C["kvW"][a]
        g = glob_rows[m]
        k2 = r + 2
        if k2 < K:
            if g == 0:
                bands[:, k2, m] -= C["kh0"]
                bandL[k2, m] += C["c00"]
                bandR[k2, m] += C["c0W"]
            if g == Hout - 1:
                bands[:, k2, m] -= C["khb"]
                bandL[k2, m] += C["cH0"]
                bandR[k2, m] += C["cHW"]
    for k in zero_rows:
        bands[:, k, :] = 0.0
        bandL[k, :] = 0.0
        bandR[k, :] = 0.0
    return bands, bandL, bandR


def _pack(bands):
    """[5, K, M] -> [K, 5*M] matching lhsT slices [K, t*M:(t+1)*M]."""
    b, K, M = bands.shape
    return np.ascontiguousarray(
        bands.transpose(1, 0, 2).reshape(K, b * M).astype(np.float32)
    )


def _scatter_m128(bands, bandL, bandR, Mz):
    """Place dense output cols into the 128-wide layout: even rows at
    partitions 0..h-1, odd rows at 64..64+h-1 (rest zero)."""
    h = Mz // 2
    K = bands.shape[1]
    b128 = np.zeros((5, K, 128), dtype=np.float64)
    l128 = np.zeros((K, 128), dtype=np.float64)
    r128 = np.zeros((K, 128), dtype=np.float64)
    for m in range(Mz):
        col = m if m < h else 64 + (m - h)
        b128[:, :, col] = bands[:, :, m]
        l128[:, col] = bandL[:, m]
        r128[:, col] = bandR[:, m]
    return b128, l128, r128


def _core_weights(c, w1, w2, w3, w4):
    """All weight arrays for core c, keyed by DRAM tensor name."""
    CA = _stage_consts(w1, w2)
    CB = _stage_consts(w3, w4)
    out = {}
    far = np.full(BM, 10 ** 9)  # glob rows that trigger no edits

    # --- stage A ---
    def bands_a(blk):
        Mz = min(BM, NZ - BM * blk)
        Kx = Mz + 4
        rm = _rowmap_permuted(Mz)
        glob = np.array([RPC * c - 4 + BM * blk + rm[m] for m in range(Mz)])
        return _scatter_m128(*_build_stage_bands(CA, Kx, Mz, rm, glob, H), Mz)

    gen = _scatter_m128(
        *_build_stage_bands(CA, 128, BM, _rowmap_permuted(BM), far, H), BM
    )
    blk0 = bands_a(0)
    blk8 = bands_a(NBLK - 1)
    out["wa"], out["wla"], out["wra"] = _pack(gen[0]), *(
        np.ascontiguousarray(g.astype(np.float32)) for g in gen[1:]
    )
    out["wa0"], out["wla0"], out["wra0"] = _pack(blk0[0]), *(
        np.ascontiguousarray(g.astype(np.float32)) for g in blk0[1:]
    )
    out["wa8"], out["wla8"], out["wra8"] = _pack(blk8[0]), *(
        np.ascontiguousarray(g.astype(np.float32)) for g in blk8[1:]
    )

    # --- stage B ---
    def bands_b(d):
        Mo = min(BM, OPC - BM * d)
        Kp = Mo + 4
        rm = np.arange(Mo)
        glob = np.array([OPC * c + BM * d + m for m in range(Mo)])
        zero = [
            k
            for k in range(Kp)
            if not (0 <= OPC * c - 2 + BM * d + k < H // 2)
        ]
        return _build_stage_bands(CB, Kp, Mo, rm, glob, H // 2, zero)

    genb = _build_stage_bands(CB, 128, BM, np.arange(BM), far, H // 2)
    d0 = bands_b(0)
    d4 = bands_b(NBB - 1)
    out["wb"], out["wlb"], out["wrb"] = _pack(genb[0]), *(
        np.ascontiguousarray(g.astype(np.float32)) for g in genb[1:]
    )
    out["wb0"], out["wlb0"], out["wrb0"] = _pack(d0[0]), *(
        np.ascontiguousarray(g.astype(np.float32)) for g in d0[1:]
    )
    out["wb4"], out["wlb4"], out["wrb4"] = _pack(d4[0]), *(
        np.ascontiguousarray(g.astype(np.float32)) for g in d4[1:]
    )
    return out


def _local_x(x2, c):
    """[XROWS, XCOLS] zero-padded slice for core c (rows RPC*c-6 .., cols -2..)."""
    r0 = RPC * c - 6
    out = np.zeros((XROWS, XCOLS), dtype=_host_dt())
    rlo, rhi = max(r0, 0), min(r0 + XROWS, H)
    out[rlo - r0 : rhi - r0, 2 : 2 + W] = x2[rlo:rhi].astype(_host_dt())
    return out


# ---------------------------------------------------------------- program ---

_PROGRAM = {}


MM_DTYPE = "bfloat16"  # "float32" (4 cyc/row, exact), "float32r" (1 cyc/row,
#                          ~2.5e-4 rel err), or "bfloat16" (1 cyc/row, ~1e-3)


def _host_dt():
    if MM_DTYPE == "bfloat16":
        import ml_dtypes
        return ml_dtypes.bfloat16
    return np.float32


def _build_program(loop=False, variant="full", unroll=1, staggered=False):
    # variant: "full" | "mm_only" (PE stream, no drains/pool/stageB)
    #        | "a_only" (stage A incl. pool+scatter) | "dma_only"
    # unroll: bodies emitted per loop iteration (cross-body overlap, no
    #         all-engine barrier between unrolled bodies)
    import concourse.bacc as bacc
    import concourse.mybir as mybir
    import concourse.tile as tile
    from contextlib import ExitStack

    f32 = mybir.dt.float32
    mmdt = getattr(mybir.dt, MM_DTYPE)

    def mm(ap):
        return ap

    # loop: False = straight-line; True = dynamic niter via values_load;
    #       int N = static hardware loop of N iterations (for timing).
    nc = bacc.Bacc("TRN2")

    xdram = nc.dram_tensor("x", [XROWS, XCOLS], mmdt, kind="ExternalInput")
    wshape = {
        "wa": [128, 5 * 128], "wa0": [128, 5 * 128], "wa8": [44, 5 * 128],
        "wb": [128, 5 * BM], "wb0": [128, 5 * BM], "wb4": [20, 5 * 16],
        "wlb": [128, BM], "wrb": [128, BM],
        "wlb0": [128, BM], "wrb0": [128, BM],
        "wlb4": [20, 16], "wrb4": [20, 16],
        # host-precomputed stage-A L/R border corrections, one col per block
        "cal": [128, NBLK], "car": [128, NBLK],
    }
    wdram = {
        k: nc.dram_tensor(k, v, mmdt, kind="ExternalInput")
        for k, v in wshape.items()
    }
    zdram = nc.dram_tensor("zpad", [128, 4], mmdt, kind="ExternalInput")
    if loop is True:
        ndram = nc.dram_tensor("niter", [1, 1], mybir.dt.int32,
                               kind="ExternalInput")
    outdram = nc.dram_tensor("out", [OPC, WH], mmdt, kind="ExternalOutput")

    with tile.TileContext(nc) as tc:
        with (
            tc.tile_pool(name="wpool", bufs=1) as wpool,
            tc.tile_pool(name="xpool", bufs=3) as xpool,
            tc.tile_pool(name="zpool", bufs=3) as zpool,
            tc.tile_pool(name="hpool", bufs=2) as hpool,
            tc.tile_pool(name="h2pool", bufs=2) as h2pool,
            tc.tile_pool(name="spool", bufs=2) as spool,
            tc.tile_pool(name="ppool", bufs=1) as ppool,
            tc.tile_pool(name="opool", bufs=2) as opool,
            tc.tile_pool(name="zps_pool",
                         bufs=(6 if variant == "mm_outer" else 5),
                         space="PSUM") as zps_pool,
            tc.tile_pool(name="bps_pool",
                         bufs=(1 if variant == "mm_outer" else 2),
                         space="PSUM") as bps_pool,
            tc.tile_pool(name="cps_pool", bufs=1, space="PSUM") as cps_pool,
        ):
            wt = {}
            for k, shp in wshape.items():
                wt[k] = wpool.tile(shp, mmdt, name=f"wt_{k}")
                nc.sync.dma_start(out=wt[k], in_=wdram[k][:])

            if loop is True:
                import concourse.mybir as _mb
                nt = wpool.tile([1, 1], _mb.dt.int32, name="nt")
                nc.sync.dma_start(out=nt, in_=ndram[:])
                nval = nc.values_load(nt[0:1, 0:1], min_val=1, max_val=4096)
                loop_cm = tc.For_i(0, nval, 1, name="rep")
            elif isinstance(loop, int) and loop > 1:
                loop_cm = tc.For_i(0, loop, 1, name="rep",
                                   staggered_reset=staggered)
            else:
                loop_cm = None

            # pad columns never change across iterations: init outside the loop
            pooled = []
            for d in range(NBB):
                pt = ppool.tile([128, WH + 4], mmdt, name=f"pooled{d}")
                pooled.append(pt)
                nc.sync.dma_start(out=pt[:, 0:2], in_=zdram[:, 0:2])
                nc.sync.dma_start(out=pt[:, WH + 2 : WH + 4], in_=zdram[:, 2:4])

            xts = {}

            def load_x(b):
                if b in xts or b >= NBLK:
                    return
                Kx = min(BM, NZ - BM * b) + 4
                xt = xpool.tile([128, W + 4], mmdt, name="xt")
                nc.sync.dma_start(
                    out=xt[0:Kx], in_=xdram[BM * b : BM * b + Kx, :]
                )
                xts[b] = xt

            def emit_stage_a_block(b):
                Mz = min(BM, NZ - BM * b)
                h = Mz // 2
                Kx = Mz + 4
                wa_t = wt["wa0"] if b == 0 else (wt["wa8"] if b == NBLK - 1 else wt["wa"])
                d_t, po = divmod(62 * b, BM)
                xt = xts.pop(b)
                if variant == "mm_outer":
                    # tap-outer: consecutive matmuls share the stationary band
                    for lo, hi in ((0, 6), (6, 12), (12, 16)):
                        zpss = [zps_pool.tile([128, CH], f32, name="zps")
                                for _ in range(hi - lo)]
                        for t in range(5):
                            for jh, jj in enumerate(range(lo, hi)):
                                nc.tensor.matmul(
                                    zpss[jh],
                                    lhsT=mm(wa_t[0:Kx, t * 128 : (t + 1) * 128]),
                                    rhs=mm(xt[0:Kx, CH * jj + t : CH * jj + t + CH]),
                                    start=(t == 0), stop=(t == 4),
                                    skip_group_check=True,
                                )
                    return
                if variant == "mm_1024":
                    for jj in range(NCHUNK_A // 2):
                        zps = zps_pool.tile([128, 2 * CH], f32, name="zps2")
                        for t in range(5):
                            nc.tensor.matmul(
                                zps,
                                lhsT=mm(wa_t[0:Kx, t * 128 : (t + 1) * 128]),
                                rhs=mm(xt[0:Kx, 2 * CH * jj + t : 2 * CH * jj + t + 2 * CH]),
                                start=(t == 0), stop=(t == 4),
                            )
                    return
                zsb = zpool.tile([128, W], mmdt, name="zsb")
                for jj in range(NCHUNK_A):
                    zps = zps_pool.tile([128, CH], f32, name="zps")
                    corr = ("L" if jj == 0 else
                            "R" if jj == NCHUNK_A - 1 else None)
                    if variant == "dma_only":
                        continue
                    for t in range(5):
                        nc.tensor.matmul(
                            zps,
                            lhsT=mm(wa_t[0:Kx, t * 128 : (t + 1) * 128]),
                            rhs=mm(xt[0:Kx, CH * jj + t : CH * jj + t + CH]),
                            start=(t == 0),
                            stop=(t == 4),
                        )
                    if variant == "mm_only":
                        continue
                    c0 = CH * jj
                    nc.scalar.copy(out=zsb[:, c0 : c0 + CH], in_=zps)
                    if corr == "L":
                        nc.vector.tensor_add(
                            out=zsb[:, 0:1], in0=zsb[:, 0:1],
                            in1=wt["cal"][:, b : b + 1],
                        )
                    elif corr == "R":
                        nc.vector.tensor_add(
                            out=zsb[:, W - 1 : W], in0=zsb[:, W - 1 : W],
                            in1=wt["car"][:, b : b + 1],
                        )
                if variant == "mm_only":
                    return
                hp = hpool.tile([128, WH], mmdt, name="hp")
                if variant == "dma_only":
                    st = spool.tile([64, WH], mmdt, name="stg")
                    nc.sync.dma_start(
                        out=pooled[d_t][po : po + h, 2 : 2 + WH], in_=st[0:h]
                    )
                    return
                # pool + scatter in half-width pieces so the first scatter
                # lands while the second half is still reducing (shortens the
                # chain stage-B joins wait on)
                hp2 = h2pool.tile([64, WH], mmdt, name="hp2")
                st = spool.tile([64, WH], mmdt, name="stg")
                for piece in range(2):
                    cl, cr = piece * (WH // 2), (piece + 1) * (WH // 2)
                    nc.vector.tensor_max(
                        out=hp[:, cl:cr],
                        in0=zsb[:, 2 * cl : 2 * cr : 2],
                        in1=zsb[:, 2 * cl + 1 : 2 * cr : 2],
                    )
                    # row-pair max: TT needs equal base partitions, so first
                    # move the odd-row half down to base 0
                    nc.vector.tensor_copy(out=hp2[:, cl:cr], in_=hp[64:128, cl:cr])
                    nc.vector.tensor_max(
                        out=st[0:h, cl:cr], in0=hp[0:h, cl:cr], in1=hp2[0:h, cl:cr]
                    )
                    # scatter the block's pooled rows into the stage-B input
                    # tiles (DMA: compute engines can't shift partitions)
                    nc.sync.dma_start(
                        out=pooled[d_t][po : po + h, 2 + cl : 2 + cr],
                        in_=st[0:h, cl:cr],
                    )
                    if b >= 2 and b % 2 == 0:
                        # pooled tiles overlap by 4 rows at band boundaries
                        nc.sync.dma_start(
                            out=pooled[b // 2 - 1][BM : BM + 4, 2 + cl : 2 + cr],
                            in_=st[0:4, cl:cr],
                        )

            def emit_stage_b_block(d):
                Mo = min(BM, OPC - BM * d)
                Kp = Mo + 4
                wb_t = wt["wb0"] if d == 0 else (wt["wb4"] if d == NBB - 1 else wt["wb"])
                wlb_t = wt["wlb0"] if d == 0 else (wt["wlb4"] if d == NBB - 1 else wt["wlb"])
                wrb_t = wt["wrb0"] if d == 0 else (wt["wrb4"] if d == NBB - 1 else wt["wrb"])
                for half in range(2):
                    osb = opool.tile([BM, WH // 2], mmdt, name="osb")
                    for jh in range(NCHUNK_B // 2):
                        jj = half * (NCHUNK_B // 2) + jh
                        bps = bps_pool.tile([BM, CH], f32, name="bps")
                        corr = ("L" if jj == 0 else
                                "R" if jj == NCHUNK_B - 1 else None)
                        if variant == "dma_only":
                            continue
                        for t in range(5):
                            nc.tensor.matmul(
                                bps[0:Mo],
                                lhsT=mm(wb_t[0:Kp, t * Mo : (t + 1) * Mo]),
                                rhs=mm(pooled[d][0:Kp, CH * jj + t : CH * jj + t + CH]),
                                start=(t == 0),
                                stop=(t == 4),
                            )
                        if variant in ("mm_only", "mm_outer", "mm_1024"):
                            continue
                        cps = None
                        if corr == "L":
                            # rhs col0 = zero pad col, col1 = pooled col 0
                            cps = cps_pool.tile([128, 2], f32, name="cpsb", tag="cps")
                            nc.tensor.matmul(
                                cps[0:Mo], lhsT=mm(wlb_t[0:Kp, 0:Mo]),
                                rhs=mm(pooled[d][0:Kp, 1:3]), start=True, stop=True,
                            )
                        elif corr == "R":
                            # rhs col0 = pooled col WH-1, col1 = zero pad col
                            cps = cps_pool.tile([128, 2], f32, name="cpsb", tag="cps")
                            nc.tensor.matmul(
                                cps[0:Mo], lhsT=mm(wrb_t[0:Kp, 0:Mo]),
                                rhs=mm(pooled[d][0:Kp, WH + 1 : WH + 3]), start=True, stop=True,
                            )
                        nc.scalar.copy(
                            out=osb[0:Mo, CH * jh : CH * (jh + 1)], in_=bps[0:Mo]
                        )
                        if corr == "L":
                            nc.vector.tensor_add(
                                out=osb[0:Mo, 0:1], in0=osb[0:Mo, 0:1],
                                in1=cps[0:Mo, 1:2],
                            )
                        elif corr == "R":
                            nc.vector.tensor_add(
                                out=osb[0:Mo, WH // 2 - 1 : WH // 2],
                                in0=osb[0:Mo, WH // 2 - 1 : WH // 2],
                                in1=cps[0:Mo, 0:1],
                            )
                    if variant not in ("mm_only", "mm_outer", "mm_1024"):
                        nc.sync.dma_start(
                            out=outdram[BM * d : BM * d + Mo,
                                        (WH // 2) * half : (WH // 2) * (half + 1)],
                            in_=osb[0:Mo],
                        )

            with (loop_cm if loop_cm is not None else ExitStack()):
                # Interleave: B-block d only needs A-blocks <= 2d+2, so run it
                # as soon as its pooled tile is complete. B's ACT/DMA work
                # then overlaps A's PE-bound stream instead of serializing.
                # B-block d is ready after A-block 2d+2's pool chain; emit it
                # one A-block later still so the chain has a full block of
                # slack before the PE reaches B's matmuls. Hold B2 back to
                # cover A8's drain (B3/B4 are forced to the tail anyway).
                do_b = variant != "a_only"
                for _u in range(unroll):
                    load_x(0)
                    for b in range(NBLK):
                        load_x(b + 1)
                        emit_stage_a_block(b)
                        if do_b and b in (3, 5):
                            emit_stage_b_block((b - 1) // 2 - 1)
                    if do_b:
                        for d in (2, 3, 4):
                            emit_stage_b_block(d)

    nc.compile()
    return nc


def get_program(loop=False, variant="full", unroll=1, staggered=False):
    key = f"nc_{loop}_{variant}_{unroll}_{staggered}"
    if key not in _PROGRAM:
        _PROGRAM[key] = _build_program(loop=loop, variant=variant,
                                       unroll=unroll, staggered=staggered)
    return _PROGRAM[key]


def build_in_maps(x2, w1, w2, w3, w4):
    in_maps = []
    hdt = _host_dt()
    for c in range(NCORES):
        m = {"x": _local_x(x2, c), "zpad": np.zeros((128, 4), hdt)}
        cw = _core_weights(c, w1, w2, w3, w4)
        # stage-A L/R border corrections, evaluated on host (f64) instead of
        # on-device single-column matmuls
        r0 = RPC * c - 6
        xcol = np.zeros((XROWS, 2), np.float64)
        rlo, rhi = max(r0, 0), min(r0 + XROWS, H)
        xcol[rlo - r0 : rhi - r0, 0] = x2[rlo:rhi, 0]
        xcol[rlo - r0 : rhi - r0, 1] = x2[rlo:rhi, W - 1]
        cal = np.zeros((128, NBLK), np.float64)
        car = np.zeros((128, NBLK), np.float64)
        for b in range(NBLK):
            Kx = min(BM, NZ - BM * b) + 4
            wl = cw["wla0"] if b == 0 else (cw["wla8"] if b == NBLK - 1 else cw["wla"])
            wr = cw["wra0"] if b == 0 else (cw["wra8"] if b == NBLK - 1 else cw["wra"])
            cal[:, b] = wl.astype(np.float64).T @ xcol[BM * b : BM * b + Kx, 0]
            car[:, b] = wr.astype(np.float64).T @ xcol[BM * b : BM * b + Kx, 1]
        m["cal"] = np.ascontiguousarray(cal.astype(hdt))
        m["car"] = np.ascontiguousarray(car.astype(hdt))
        for k, v in cw.items():
            if k.startswith("wla") or k.startswith("wra"):
                continue
            m[k] = np.ascontiguousarray(v.astype(hdt))
        in_maps.append(m)
    return in_maps


def kernel(x, w1, w2, w3, w4, H=None, W=None, nTh=None, nTw=None, **_):
    from concourse.bass_utils import run_bass_kernel_spmd

    x2 = np.asarray(x, dtype=np.float32).reshape(8192, 8192)
    ws = [np.asarray(w, dtype=np.float32).reshape(3, 3) for w in (w1, w2, w3, w4)]
    nc = get_program()
    in_maps = build_in_maps(x2, *ws)
    res = run_bass_kernel_spmd(nc, in_maps, core_ids=list(range(NCORES)))
    out = np.concatenate([res.results[c]["out"] for c in range(NCORES)], axis=0)
    return out.reshape(1, 1, 4096, 4096).astype(np.float32)

